# revision 119
# baseline (speedup 1.0000x reference)
"""Causal multi-head attention (qkv proj + attention + out proj) on 8 TRN2 cores.

Problem: x[2,2048,512] -> qkv proj (w_qkv [512,1536]) -> 8 heads x 64 dim causal
attention -> out proj (w_out [512,512] + b_out). Key-padding mask is all-ones
per the problem spec, so only the causal mask is applied.

Sharding: data-parallel over batch (2) x tensor-parallel over heads (4 groups
of 2 heads).  Core c handles batch c//4 and heads {2*(c%4), 2*(c%4)+1}.  Each
core computes its 2 heads' partial out-projection [N, DIM] in fp32; the host
sums the 4 partials per batch and adds b_out (the unshard step for TP-partial
outputs).

Per-core kernel (all activations/weights fp16: a 10-bit mantissa for
~N(0,1) data at full PE rate; rel err ~5e-4 vs the fp32 reference):
  - x arrives pre-transposed ([d, n] fp16) and weights partition-major
    packed, so every DMA is a plain >=1KB-descriptor copy on one queue
    (copy<->xbar-transpose transitions serialize globally on the DGE, and
    elements under 512B pay 2x) and the TensorE runs no input transposes.
  - q^T/k^T [128, N] via w-chunk matmuls; v computed directly in [j, dh]
    orientation (lhsT = xT j-tile, rhs = wv) -- no v transpose.  Block 0
    projects q/k in column halves with half-width dots/exp interleaved so
    the first exp fires as soon as wq, wk and x[:, 0:256] land.
  - Attention per (i-block g, j-chunk c): both heads' dots land in one
    2-bank PSUM pair tile; ONE exp (ScalarE, scale=1/8) covers both heads
    (ScalarE is the bottleneck engine at ~36.4us busy -- ~29us of raw
    columns + ~185ns/instr access overhead, halved by the head pairing);
    causal diagonal handled by a single paired affine_select on GpSimd.
  - P@V runs as sequential per-(head, i-subtile) PSUM chains (one open
    accumulation group per 2KB bank is the hw limit) rotating through 2
    banks, consuming retained pb tiles one block behind the dots; a shared
    ones-column yields row sums; normalization = fast reciprocal +
    tensor_scalar_mul on DVE; oh tiles are PE-transposed (53ns) into ohT
    for the out-projection, whose fp16 stores ride DVE copies mid-kernel
    and ScalarE copies on the tail (so tail norms never queue).
  - Emission is software-pipelined with a dummy-matmul warmup burning the
    input-DMA wait (the cost model halves PE speed for its first ~3us):
    block g's chunks carry block g+1's q/k, block g's v, block g-1's P@V
    chains and older out-projections as evenly-paced spread filler; block
    3's P@V chains inject as soon as their diagonal probs exist, and the
    tail keeps only s3's chain + two output tiles on the critical path.
"""

import numpy as np

B, N, DIM = 2, 2048, 512
HEADS, DH = 8, 64
SCALE = DH ** -0.5
NT = N // 128      # 16 row tiles
NB = N // 512      # 4 blocks
CC = DIM // 128    # 4 contraction chunks
NCORES = 8

_cache = {}


def _build():
    import concourse.bass as bass
    import concourse.mybir as mybir
    import concourse.tile as tile
    from concourse import bacc
    from contextlib import ExitStack

    F16 = mybir.dt.float16
    F32 = mybir.dt.float32
    Exp = mybir.ActivationFunctionType.Exp

    nc = bacc.Bacc()
    # x arrives pre-transposed ([d, n]) -- the tensor-parallel shard layout
    # is chosen host-side, and [d, n] is what every consumer contracts over.
    xT_d = nc.declare_dram_parameter("xT", [DIM, N], F16, isOutput=False).ap()
    # wq and [wk | wv] (this core's 128 head-dims each), partition-major
    # packed host-side so DMA descriptors move >=1KB contiguous; q split
    # from k/v so it lands before the rest of the input stream
    wq_d = nc.declare_dram_parameter("wq", [128, DIM], F16, isOutput=False).ap()
    wkv_d = nc.declare_dram_parameter("wkv", [128, 2 * DIM], F16, isOutput=False).ap()
    wo_d = nc.declare_dram_parameter("wo", [128, DIM], F16, isOutput=False).ap()
    out_d = nc.declare_dram_parameter("out", [N, DIM], F16, isOutput=True).ap()

    with tile.TileContext(nc) as tc:
        with ExitStack() as ctx:
            persist = ctx.enter_context(tc.tile_pool(name="persist", bufs=1))

            xT = persist.tile([128, CC, N], F16, tag="xT")
            wqkv_sb = persist.tile([128, 3, CC, 128], F16, tag="wqkv")
            wo_sb = persist.tile([128, DIM], F16, tag="wo")
            qT2 = persist.tile([128, N], F16, tag="qT2")
            kT2 = persist.tile([128, N], F16, tag="kT2")
            # vo: [v_h0 (0:64) | ones (64) | v_h1 (65:129)] -- ones shared.
            vo = persist.tile([128, NT, 129], F16, tag="vo")
            nc.vector.memset(vo[:, :, 64:65], 1.0)  # v copies fill the rest
            warm = persist.tile([128, 128], F16, tag="warm")
            nc.vector.memset(warm, 0.5)
            # identity for the (tiny) PE transposes of the head outputs
            id16 = persist.tile([128, 128], F16, tag="id16")
            nc.vector.memset(id16, 0.0)
            nc.gpsimd.affine_select(
                out=id16, in_=id16, compare_op=mybir.AluOpType.not_equal,
                fill=1.0, base=0, pattern=[[-1, 128]], channel_multiplier=1)


            pools = [
                tc.tile_pool(name="dp", bufs=2, space="PSUM"),   # dots pairs
                tc.tile_pool(name="av", bufs=2, space="PSUM"),   # P@V accum
                tc.tile_pool(name="pj", bufs=2, space="PSUM"),   # qkv/outproj
                tc.tile_pool(name="pb", bufs=30),                # probs pairs
                tc.tile_pool(name="oh", bufs=4),
                tc.tile_pool(name="ohT", bufs=16),
                tc.tile_pool(name="sm", bufs=8),
            ]
            (dp_pool, av_pool, pj_pool, pb_pool, oh_pool, ohT_pool,
             sm_pool) = [ctx.enter_context(p) for p in pools]

            # --- input DMAs.  All plain copies on one queue (transitions
            # between copy- and transpose-mode DMAs serialize globally on the
            # DGE, so the kernel uses no DMA transposes at all).  First x
            # slices are halved so qkv(0) starts early. ---
            def x_dma(n0, n1):
                nc.sync.dma_start(
                    out=xT[:, :, n0:n1],
                    in_=xT_d[:, n0:n1].rearrange("(c p) n -> p c n", p=128))

            x_dma(0, 256)
            nc.sync.dma_start(
                out=wqkv_sb[:, 0, :, :],
                in_=wq_d.rearrange("p (c d) -> p c d", c=CC))
            nc.sync.dma_start(
                out=wqkv_sb[:, 1:3, :, :],
                in_=wkv_d.rearrange("p (w c d) -> p w c d", w=2, c=CC))
            x_dma(256, 512)
            nc.sync.dma_start(out=wo_sb, in_=wo_d)
            for g in range(1, NB):
                x_dma(g * 512, (g + 1) * 512)

            # p-state warmup: the cost model runs the PE at half speed until
            # ~3us after it first goes busy, so burn the input-DMA wait on
            # dummy matmuls (they retire before the first projection).
            for i in range(14):
                wt = pj_pool.tile([128, 128], F32, tag="pj", name="wrm")
                nc.tensor.matmul(out=wt, lhsT=warm, rhs=warm,
                                 start=True, stop=True)

            def qkv_ops(g, include_qk=True):
                """Closures projecting q/k (dh-major) and v (j-major), block g."""
                ops = []
                state = {}

                def mk_qk(key, wi, dst):
                    def mm(c):
                        def f():
                            if c == 0:
                                state[key] = pj_pool.tile(
                                    [128, 512], F32, tag="pj", name=f"p_{key}")
                            nc.tensor.matmul(
                                out=state[key],
                                lhsT=wqkv_sb[:, wi, c, :],
                                rhs=xT[:, c, g * 512:(g + 1) * 512],
                                start=(c == 0), stop=(c == CC - 1))
                        return f

                    def cp():
                        nc.vector.tensor_copy(
                            out=dst[:, g * 512:(g + 1) * 512],
                            in_=state.pop(key))
                    return [mm(c) for c in range(CC)] + [cp]

                if include_qk:
                    ops += mk_qk("q", 0, qT2)
                    ops += mk_qk("k", 1, kT2)
                    return ops

                def mk_v(t, c):
                    def f():
                        if t == 0 and c == 0:
                            state["v"] = pj_pool.tile(
                                [128, 4, 128], F32, tag="pj", name="p_v")
                        nc.tensor.matmul(
                            out=state["v"][:, t, :],
                            lhsT=xT[:, c, (4 * g + t) * 128:(4 * g + t + 1) * 128],
                            rhs=wqkv_sb[:, 2, c, :],
                            start=(c == 0), stop=(c == CC - 1))
                    return f

                def mk_vcp(t, h):
                    def f():
                        lo = 0 if h == 0 else 65
                        nc.vector.tensor_copy(
                            out=vo[:, 4 * g + t, lo:lo + 64],
                            in_=state["v"][:, t, h * 64:h * 64 + 64])
                        if t == 3 and h == 1:
                            state.pop("v")
                    return f

                for t in range(4):
                    ops += [mk_v(t, c) for c in range(CC)]
                for t in range(4):
                    ops += [mk_vcp(t, 0), mk_vcp(t, 1)]
                return ops

            def outproj_ops(g, ohT_box, s_list=(0, 1, 2, 3), st_act=False):
                """Closures for block g's out-projection rows. ohT_box holds
                per-s ohT tiles, dereferenced at closure-run time.  st_act
                routes the PSUM->SBUF copy through the Scalar engine -- used
                only for the final tiles, after the last exp has retired, so
                the tail norms don't queue behind copies on the DVE."""
                ops = []
                state = {}

                def mk(s):
                    t = g * 4 + s

                    def mm():
                        state[s] = pj_pool.tile(
                            [128, DIM], F32, tag="pj", name="p_o")
                        nc.tensor.matmul(
                            out=state[s], lhsT=ohT_box[s], rhs=wo_sb,
                            start=True, stop=True)

                    def st():
                        stt = sm_pool.tile([128, DIM], F16, tag="st", name="st")
                        if st_act:
                            nc.scalar.copy(out=stt, in_=state.pop(s))
                        else:
                            nc.vector.tensor_copy(out=stt, in_=state.pop(s))
                        nc.sync.dma_start(
                            out=out_d[t * 128:(t + 1) * 128, :], in_=stt)
                    return [mm, st]

                for s in s_list:
                    ops.extend(mk(s))
                return ops

            def av_ops(g, pbs, ohT_box, s_list=(0, 1, 2, 3), state=None):
                """Closures for block g's P@V: sequential per-(h,s) chains
                rotating through the 2 av PSUM banks, each ending in its
                normalization.  After each s-pair completes, its oh half is
                DMA-transposed into ohT_box (so out-projections can start
                before the second pair finishes).  `pbs` is read at closure
                run time, so it may still be filling when ops are built;
                pass `state` to share oh halves across split s_list calls."""
                ops = []
                state = {} if state is None else state

                def mk_start(h, s):
                    def f():
                        if ("oh", s // 2) not in state:
                            state[("oh", s // 2)] = oh_pool.tile(
                                [128, 2, 128], F16, tag="oh", name="oh")
                        state[(h, s)] = av_pool.tile(
                            [128, 65], F32, tag="av", name="av")
                    return f

                def mk_mm(h, s, c):
                    v_lo = 0 if h == 0 else 64

                    def f():
                        nc.tensor.matmul(
                            out=state[(h, s)],
                            lhsT=pbs[c][:, h, s * 128:(s + 1) * 128],
                            rhs=vo[:, c, v_lo:v_lo + 65],
                            start=(c == 0), stop=(c == 4 * g + s))
                    return f

                def mk_norm(h, s):
                    def f():
                        av = state.pop((h, s))
                        oh_p = state[("oh", s // 2)]
                        rec = sm_pool.tile([128, 1], F32, tag="rec", name="rec")
                        sum_sl = av[:, 64:65] if h == 0 else av[:, 0:1]
                        osl = av[:, 0:64] if h == 0 else av[:, 1:65]
                        nc.vector.reciprocal_approx_fast(out=rec, in_=sum_sl)
                        nc.vector.tensor_scalar_mul(
                            oh_p[:, s % 2, h * 64:(h + 1) * 64], osl, rec)
                        if h == 1:
                            # transpose this s-tile into its ohT slot (PE
                            # transpose is ~53ns; no DMA sem hops)
                            ohT_b = ohT_pool.tile(
                                [128, 128], F16, tag="ohT", name="ohT")
                            pt = pj_pool.tile([128, 128], F16, tag="pj",
                                              name="pt")
                            nc.tensor.transpose(
                                out=pt, in_=oh_p[:, s % 2, :], identity=id16)
                            nc.vector.tensor_copy(out=ohT_b, in_=pt)
                            ohT_box.append(ohT_b)
                            if s % 2 == 1:
                                state.pop(("oh", s // 2))
                    return f

                for s in s_list:
                    for h in (0, 1):
                        ops.append(mk_start(h, s))
                        ops += [mk_mm(h, s, c) for c in range(4 * g + s + 1)]
                        ops.append(mk_norm(h, s))
                return ops

            def emit_chunk(g, c, pbs):
                """Dots pair + exp (+ diagonal mask) for (block g, chunk c)."""
                r = c - 4 * g
                lo = 128 * r if r > 0 else 0
                dp = dp_pool.tile([128, 2, 512], F32, tag="dp", name="dp")
                for h in (0, 1):
                    hb = h * 64
                    nc.tensor.matmul(
                        out=dp[:, h, lo:512],
                        lhsT=kT2[hb:hb + 64, c * 128:(c + 1) * 128],
                        rhs=qT2[hb:hb + 64, g * 512 + lo:(g + 1) * 512],
                        start=True, stop=True)
                pbt = pb_pool.tile([128, 2, 512], F16, tag="pb", name="pb")
                nc.scalar.activation(
                    out=pbt[:, :, lo:512], in_=dp[:, :, lo:512],
                    func=Exp, scale=SCALE)
                if r >= 0:
                    # zero j > i on the diagonal 128-tile, both heads
                    nc.gpsimd.affine_select(
                        out=pbt[:, :, lo:lo + 128],
                        in_=pbt[:, :, lo:lo + 128],
                        compare_op=mybir.AluOpType.is_ge,
                        fill=0.0, base=0, pattern=[[0, 2], [1, 128]],
                        channel_multiplier=-1)
                pbs.append(pbt)

            def emit_attn_block(g, spread, late=None, pbs_out=None,
                                next_pre=None, skip=0):
                """Both heads' dots/exp/mask for i-block g; returns pb tiles.
                `late` maps chunk index -> extra closures appended to the
                spread once that chunk has been emitted.  `next_pre` =
                (g+1, pbs_next, count): emit the next block's first chunks
                interleaved with this block's last ones so the exp stream
                has no boundary hole.  `skip` chunks were already emitted."""
                spread = list(spread)
                late = late or {}
                nch = 4 * g + 4
                pbs = pbs_out if pbs_out is not None else []
                for c in range(skip, nch):
                    emit_chunk(g, c, pbs)
                    if next_pre is not None and c >= nch - next_pre[2]:
                        emit_chunk(next_pre[0], c - (nch - next_pre[2]),
                                   next_pre[1])
                    spread.extend(late.pop(c, ()))
                    # spread remaining filler evenly over remaining chunks
                    for _ in range(-(-len(spread) // (nch - c))):
                        if spread:
                            spread.pop(0)()
                for op in spread:
                    op()
                return pbs

            def emit_block0():
                """Block 0 bespoke: q/k projected in column halves with
                half-width dots/exp interleaved, so the first exp fires as
                soon as wq, wk and x[:, 0:256] have landed."""
                st = {}
                pbs = []

                def qk_chain(key, wi, dst, p0, p1, cp_act=False):
                    for c in range(CC):
                        if c == 0 and p0 == 0:
                            st[key] = pj_pool.tile(
                                [128, 512], F32, tag="pj", name=f"p_{key}")
                        nc.tensor.matmul(
                            out=st[key][:, p0:p1],
                            lhsT=wqkv_sb[:, wi, c, :],
                            rhs=xT[:, c, p0:p1],
                            start=(c == 0), stop=(c == CC - 1))
                    if cp_act:
                        # ScalarE is idle pre-first-exp; don't serialize the
                        # two startup copies on the DVE
                        nc.scalar.copy(out=dst[:, p0:p1], in_=st[key][:, p0:p1])
                    else:
                        nc.vector.tensor_copy(
                            out=dst[:, p0:p1], in_=st[key][:, p0:p1])
                    if p1 == 512:
                        st.pop(key)

                def dots_half(c, p0, p1):
                    lo = 128 * c
                    pl = max(lo, p0)
                    if pl >= p1:
                        return
                    if c not in st:
                        st[c] = dp_pool.tile(
                            [128, 2, 512], F32, tag="dp", name="dp")
                        pbs.append(pb_pool.tile(
                            [128, 2, 512], F16, tag="pb", name="pb"))
                    dp, pbt = st[c], pbs[c]
                    for h in (0, 1):
                        hb = h * 64
                        nc.tensor.matmul(
                            out=dp[:, h, pl:p1],
                            lhsT=kT2[hb:hb + 64, c * 128:(c + 1) * 128],
                            rhs=qT2[hb:hb + 64, pl:p1],
                            start=True, stop=True)
                    nc.scalar.activation(
                        out=pbt[:, :, pl:p1], in_=dp[:, :, pl:p1],
                        func=Exp, scale=SCALE)
                    if p0 <= lo < p1:
                        nc.gpsimd.affine_select(
                            out=pbt[:, :, lo:lo + 128],
                            in_=pbt[:, :, lo:lo + 128],
                            compare_op=mybir.AluOpType.is_ge,
                            fill=0.0, base=0, pattern=[[0, 2], [1, 128]],
                            channel_multiplier=-1)

                qk_chain("q", 0, qT2, 0, 256)
                qk_chain("k", 1, kT2, 0, 256)
                # the q B-half chain is ready (x[256:512]) before the kA copy
                # lands, so it runs in the dotsA dependency shadow
                qk_chain("q", 0, qT2, 256, 512)
                dots_half(0, 0, 256)
                dots_half(1, 0, 256)
                # chunks 0/1 use k tiles 0/1 (in the A half), so their B dots
                # need only the q B-half; k's B half follows while exp runs
                dots_half(0, 256, 512)
                dots_half(1, 256, 512)
                qk_chain("k", 1, kT2, 256, 512)
                dots_half(2, 256, 512)
                dots_half(3, 256, 512)
                for c in (0, 1, 2, 3):
                    st.pop(c, None)
                return pbs

            def interleave(a, b, na=2, nb=1):
                """Round-robin na ops of a : nb ops of b."""
                a, b, out = list(a), list(b), []
                while a or b:
                    for _ in range(na):
                        if a:
                            out.append(a.pop(0))
                    for _ in range(nb):
                        if b:
                            out.append(b.pop(0))
                return out

            # --- software-pipelined emission ---
            # block g pipeline: qkv(g) during g-1, dots/exp(g) during g,
            # P@V+norm(g) during g+1, out-projection(g) right behind its
            # s-pair transposes.  Late blocks are exp(ScalarE)-bound, so
            # out-projections of blocks 1/2 and block 3's own early P@V
            # chains are packed into block 3 where the PE has slack.
            # v projections are only needed by the NEXT block's P@V chains,
            # and k(g) only by block g's diagonal chunks, so both ride the
            # spread; each block's dots wait only on its q copy.
            boxes = [[] for _ in range(NB)]
            pbs = emit_block0()
            qk1 = qkv_ops(1)
            for op in qk1[:5]:       # q(1) chain + copy
                op()
            sp1 = (qk1[5:] + qkv_ops(0, include_qk=False)
                   + interleave(qkv_ops(2), av_ops(0, pbs, boxes[0]), 2, 3))
            pbs = emit_attn_block(1, sp1)
            sp2 = (qkv_ops(1, include_qk=False)
                   + interleave(qkv_ops(3),
                                av_ops(1, pbs, boxes[1])
                                + outproj_ops(0, boxes[0])
                                + outproj_ops(1, boxes[1]), 2, 3))
            pbs = emit_attn_block(2, sp2)
            sp3 = (qkv_ops(2, include_qk=False)
                   + interleave(qkv_ops(3, include_qk=False),
                                av_ops(2, pbs, boxes[2])
                                + outproj_ops(2, boxes[2]), 2, 3))
            # Block 3's P@V chains slide in as early as their pb tiles exist
            # (sliced so no closure touches a chunk that isn't emitted yet),
            # and out-projections for s0-s2 leave the tail critical path.
            pbs3, st3 = [], {}
            s0 = av_ops(3, pbs3, boxes[3], s_list=(0,), state=st3)
            s1 = av_ops(3, pbs3, boxes[3], s_list=(1,), state=st3)
            s2 = av_ops(3, pbs3, boxes[3], s_list=(2,), state=st3)
            emit_attn_block(3, sp3, late={
                12: s0,
                13: s1,
                15: s2 + outproj_ops(3, boxes[3], s_list=(0, 1), st_act=True),
            }, pbs_out=pbs3)
            s3 = av_ops(3, pbs3, boxes[3], s_list=(3,), state=st3)
            for op in s3[:18]:
                op()
            for op in outproj_ops(3, boxes[3], s_list=(2,), st_act=True):
                op()
            for op in s3[18:]:
                op()
            for op in outproj_ops(3, boxes[3], s_list=(3,), st_act=True):
                op()
    nc.compile()
    return nc


def _get_nc():
    if "nc" not in _cache:
        _cache["nc"] = _build()
    return _cache["nc"]


def _in_maps(x, w_qkv, w_out):
    maps = []
    for c in range(NCORES):
        b = c // 4
        h0 = 2 * (c % 4)
        cols = slice(h0 * DH, (h0 + 2) * DH)  # 128 contiguous head cols
        def pack(w):  # [512, 128] -> [128, (c d)]: row p holds w[c*128+p, :]
            return np.ascontiguousarray(
                w.reshape(4, 128, 128).transpose(1, 0, 2).reshape(128, 512))

        wq = pack(w_qkv[:, 0:512][:, cols])
        wk = pack(w_qkv[:, 512:1024][:, cols])
        wv = pack(w_qkv[:, 1024:1536][:, cols])
        maps.append({
            "xT": np.ascontiguousarray(x[b].T).astype(np.float16),
            "wq": wq.astype(np.float16),
            "wkv": np.concatenate([wk, wv], axis=1).astype(np.float16),
            "wo": np.ascontiguousarray(w_out[cols, :]).astype(np.float16),
        })
    return maps


def _combine(results, b_out):
    out = np.zeros((B, N, DIM), np.float32)
    for c in range(NCORES):
        out[c // 4] += np.asarray(results[c]["out"], dtype=np.float32)
    out += b_out.astype(np.float32)
    return out


def kernel(**inputs):
    x = np.asarray(inputs["x"], dtype=np.float32)
    w_qkv = np.asarray(inputs["w_qkv"], dtype=np.float32)
    w_out = np.asarray(inputs["w_out"], dtype=np.float32)
    b_out = np.asarray(inputs["b_out"], dtype=np.float32)
    # inputs["mask"] is all-ones per the problem spec (key padding no-op).
    from concourse.bass_utils import run_bass_kernel_spmd
    nc = _get_nc()
    res = run_bass_kernel_spmd(nc, _in_maps(x, w_qkv, w_out), list(range(NCORES)))
    return _combine(res.results, b_out)
